# revision 51
# baseline (speedup 1.0000x reference)
"""DIoU regression loss on 8 Trainium2 NeuronCores (data-parallel).

loss = sum(1 - clip(diou(pred_i, gt_i), -1, 1)) / (N + 1e-4) over N=4M boxes.

Sharding: each core gets a contiguous slab of R = 128*T*K rows; the last
core's slab is padded with identical unit boxes whose diou == 1, so padded
rows contribute 0 to sum(1 - diou).

Layout: the host packs the 7 used columns of both boxes into a plane-major
bf16 array [14, RCORE] per core, so every on-chip operand is a unit-stride
[P, T] (or [P, 2, T]) bf16 slice -> DVE runs in its 2x packed mode and DMA
traffic is 14 MB/core instead of 36 MB.

Math (equivalent to the det3d corner-based reference): with full extents
  Ex = w*cos(r) + l*sin(r), Ey = l*cos(r) - w*sin(r), Ez = h
and per dim u = Ep+Eg, v = Ep-Eg, dc = cg-cp, g = max(|v|, |2dc|):
  2*inter_d = relu(u - g),   2*outer_d = u + g  (relu'd for y)
so with I = prod(relu(u-g)) = 8*inter_vol, U8 = 8*(volp+volg) - I = 8*union,
O4 = sum((u+g)^2) = 4*outer_diag, D = sum(dc^2):
  -diou = (4*D*U8 - I*O4) / (U8*O4)
Each core emits per-partition per-tile sums of clip(-diou, -1, 1); the host
combines: loss = (NPAD + total_negdiou) / (N + 1e-4).

Engine split: DVE does the binary chain; ACT (scalar) does sin/cos, the
abs/squares/relus and the f32<->bf16 casts around the reciprocal; the PE
row-sums the clipped tiles 0..K-2 into PSUM via ones-matmuls (drained to
SBUF during the last tile); the last tile's clipped values are DMA'd out
raw and summed by the host together with the 128 PSUM partials. GPSIMD is
deliberately unused: it shares an SBUF read port with the DVE's rd1, so any
GPSIMD streaming blocks every 2-input DVE op. sin/cos of tile k+1 are
computed during tile k's body (sn2/cs2 double-buffered) and the prefetch
DMA is deferred mid-body so it never competes with the current tile's load.
The last tile keeps its relus and the UO product on VEC so the closing
serial tail never head-of-line blocks on the ACT queue.
"""

import numpy as np
import ml_dtypes

import concourse.bacc as bacc
import concourse.mybir as mybir
import concourse.tile as tile
from concourse import bass_utils

P = 128          # SBUF partitions
T = 1304         # rows per partition per tile
K = 3            # tiles per core
NCORES = 8
RCORE = P * T * K            # 500,736 rows per core
NPAD = RCORE * NCORES        # 4,005,888
NREAL = 4_000_000
C = 14                       # planes
BF16 = mybir.dt.bfloat16
F32 = mybir.dt.float32
HALF_PI = float(np.pi / 2)

AF = mybir.ActivationFunctionType
OP = mybir.AluOpType

# plane order: w_p w_g l_p l_g h_p h_g r_p r_g x_p y_p x_g y_g z_p z_g
_PLANE_SRC = [(3, 0), (3, 1), (4, 0), (4, 1), (5, 0), (5, 1), (6, 0), (6, 1),
              (0, 0), (1, 0), (0, 1), (1, 1), (2, 0), (2, 1)]
_PLANE_PAD = np.array([1, 1, 1, 1, 1, 1, 0, 0, 0, 0, 0, 0, 0, 0],
                      dtype=np.float32)

_CACHE = {}
_TRACE = False
_LAST = None
_DEBUG = False


def _build():
    nc = bacc.Bacc("TRN2", target_bir_lowering=False, debug=False,
                   num_devices=NCORES)
    ab = nc.dram_tensor("ab", [C, RCORE], BF16, kind="ExternalInput").ap()
    out = nc.dram_tensor("out", [P, 2], F32, kind="ExternalOutput").ap()
    dumpout = nc.dram_tensor("dumpout", [P, 1, T], BF16,
                             kind="ExternalOutput").ap()
    dbg = {}
    if _DEBUG:
        for nm in ("w2", "sn2", "Ep2", "g_xy", "ti_xy", "to_xy"):
            dbg[nm] = nc.dram_tensor(f"dbg_{nm}", [P, 2, T], BF16,
                                     kind="ExternalOutput").ap()
        for nm in ("I", "U8", "negnum", "negd", "dump"):
            dbg[nm] = nc.dram_tensor(f"dbg_{nm}", [P, 1, T], BF16,
                                     kind="ExternalOutput").ap()
        for nm in ("rcp",):
            dbg[nm] = nc.dram_tensor(f"dbg_{nm}", [P, 1, T], F32,
                                     kind="ExternalOutput").ap()

    # [k][P, C, T]: plane c of tile k, partition p starts at
    # c*RCORE + k*P*T + p*T
    abv = ab.rearrange("c (k p t) -> k p c t", p=P, t=T)

    with tile.TileContext(nc) as tc:
        with (
            tc.tile_pool(name="raw", bufs=2) as rawp,
            tc.tile_pool(name="tmp", bufs=1) as tmp,
            tc.tile_pool(name="one", bufs=1) as one,
            tc.psum_pool(name="ps", bufs=1) as psp,
        ):
            psum = psp.tile([P, 1], F32, tag="psum", name="psum")
            ones = one.tile([P, 1], BF16, tag="ones", name="ones")
            nc.vector.memset(ones, 1.0)
            halfpi = one.tile([P, 1], F32, tag="halfpi", name="halfpi")
            nc.vector.memset(halfpi, HALF_PI)
            acc = one.tile([P, 2], F32, tag="acc", name="acc")

            # physical buffer tags; a tag may host several short-lived
            # logical tiles per iteration (WAR handled by the dep tracker).
            # sn2/cs2 are double-buffered for the trig software pipeline.
            _ALIAS = {
                "sn2": "A0", "cs2": "A1", "wl2": "A2", "vol2": "A3",
                "wc2": "A4", "ls2": "A5", "ws2": "A12", "lc2": "A7",
                "Ep2": "A8", "Eg2": "A9",
                "u_xy": "A4", "v_xy": "A5", "dc_xy": "A6", "g_xy": "A7",
                "a2_xy": "A13", "nv_xy": "A12", "ti_xy": "A8", "to_xy": "A9",
                "d2_xy": "A10", "o2_xy": "A11",
                "wl2": "A2", "vol2": "A3",
                "S": "B0", "u_z": "B1", "v_z": "B2", "dc_z": "B3",
                "g_z": "B4", "a2_z": "B5", "nv_z": "B5", "ti_z": "B5",
                "to_z": "B6",
                "d2_z": "B16", "o2_z": "B4", "I": "B1", "idiag": "B3",
                "O4": "B6", "U8": "B5", "IO": "B2", "DU": "B4",
                "negnum": "B7",
                "UO": "X0", "rcp": "X1", "negd": "X2", "dump": "X0",
            }
            _BUFS = {"A0": 2, "A1": 2}

            def t2(tag):
                t = _ALIAS[tag]
                return tmp.tile([P, 2, T], BF16, tag=t, name=tag,
                                bufs=_BUFS.get(t, 1))

            def t1(tag):
                t = _ALIAS[tag]
                return tmp.tile([P, 1, T], BF16, tag=t, name=tag,
                                bufs=_BUFS.get(t, 1))

            def f1(tag):
                t = _ALIAS[tag]
                return tmp.tile([P, 1, T], F32, tag=t, name=tag,
                                bufs=_BUFS.get(t, 1))

            def trig(raw_tile):
                sn2 = t2("sn2")
                cs2 = t2("cs2")
                r2 = raw_tile[:, 6:8]
                nc.scalar.activation(out=sn2, in_=r2, func=AF.Sin)
                # cos(r) = sin(pi/2 - r); keeps the arg inside the Sin
                # spline's domain.
                nc.scalar.activation(out=cs2, in_=r2, func=AF.Sin,
                                     bias=halfpi, scale=-1.0)
                return sn2, cs2

            # prologue: tile 0 DMA (sizes+trig planes first, centers second)
            raw = rawp.tile([P, C, T], BF16, tag="raw", name="raw")
            nc.sync.dma_start(out=raw[:, 0:4], in_=abv[0][:, 0:4])
            nc.sync.dma_start(out=raw[:, 4:6], in_=abv[0][:, 4:6])
            nc.sync.dma_start(out=raw[:, 6:8], in_=abv[0][:, 6:8])
            nc.sync.dma_start(out=raw[:, 8:14], in_=abv[0][:, 8:14])
            sn2, cs2 = trig(raw)

            for k in range(K):
                raw_nx = None

                w2 = raw[:, 0:2]
                l2 = raw[:, 2:4]
                h2 = raw[:, 4:6]
                cp_xy = raw[:, 8:10]
                cg_xy = raw[:, 10:12]
                zp = raw[:, 12:13]
                zg = raw[:, 13:14]

                if k == 0:
                    # tile 0: volumes first — they need no trig, so VEC works
                    # through them while the r-DMA/sin chain completes
                    wl2 = t2("wl2")
                    vol2 = t2("vol2")
                    S = t1("S")
                    nc.vector.tensor_mul(wl2, w2, l2)
                    nc.vector.tensor_mul(vol2, wl2, h2)
                    nc.vector.tensor_add(S, vol2[:, 0:1], vol2[:, 1:2])

                # --- rotated extents (VEC); sin-consumers first so the
                # products can start as soon as sin lands ---
                wc2 = t2("wc2")
                ls2 = t2("ls2")
                ws2 = t2("ws2")
                lc2 = t2("lc2")
                nc.vector.tensor_mul(ls2, l2, sn2)
                nc.vector.tensor_mul(ws2, w2, sn2)
                nc.vector.tensor_mul(wc2, w2, cs2)
                nc.vector.tensor_mul(lc2, l2, cs2)

                # --- center deltas (ACT computes |2dc| and dc^2 while VEC
                # chews on extents) ---
                dc_xy = t2("dc_xy")
                dc_z = t1("dc_z")
                nc.vector.tensor_sub(dc_xy, cg_xy, cp_xy)
                nc.vector.tensor_sub(dc_z, zg, zp)
                a2_xy = t2("a2_xy")
                a2_z = t1("a2_z")
                nc.scalar.activation(out=a2_xy, in_=dc_xy, func=AF.Abs,
                                     scale=2.0)
                nc.scalar.activation(out=a2_z, in_=dc_z, func=AF.Abs,
                                     scale=2.0)

                Ep2 = t2("Ep2")   # (Ex_p, Ey_p)
                Eg2 = t2("Eg2")
                nc.vector.tensor_add(Ep2[:, 0:1], wc2[:, 0:1], ls2[:, 0:1])
                nc.vector.tensor_sub(Ep2[:, 1:2], lc2[:, 0:1], ws2[:, 0:1])
                nc.vector.tensor_add(Eg2[:, 0:1], wc2[:, 1:2], ls2[:, 1:2])
                nc.vector.tensor_sub(Eg2[:, 1:2], lc2[:, 1:2], ws2[:, 1:2])

                # --- per-dim v first so ACT's |v| overlaps the u adds ---
                v_xy = t2("v_xy")
                v_z = t1("v_z")
                nc.vector.tensor_sub(v_xy, Ep2, Eg2)
                nc.vector.tensor_sub(v_z, h2[:, 0:1], h2[:, 1:2])
                av_xy = t2("nv_xy")
                av_z = tmp.tile([P, 1, T], BF16, tag="B18", name="av_z")
                nc.scalar.activation(out=av_xy, in_=v_xy, func=AF.Abs)
                nc.scalar.activation(out=av_z, in_=v_z, func=AF.Abs)
                u_xy = t2("u_xy")
                u_z = t1("u_z")
                nc.vector.tensor_add(u_xy, Ep2, Eg2)
                nc.vector.tensor_add(u_z, h2[:, 0:1], h2[:, 1:2])

                if k > 0:
                    # volumes as additional cover for the |v| latency
                    # (VEC-resident: GPSIMD would contend for the DVE's
                    # second SBUF read port and block every 2-input op)
                    wl2 = t2("wl2")
                    vol2 = t2("vol2")
                    S = t1("S")
                    nc.vector.tensor_mul(wl2, w2, l2)
                    nc.vector.tensor_mul(vol2, wl2, h2)
                    nc.vector.tensor_add(S, vol2[:, 0:1], vol2[:, 1:2])

                # --- g = max(|v|, |2dc|) ---
                g_xy = t2("g_xy")
                g_z = t1("g_z")
                nc.vector.tensor_tensor(out=g_xy, in0=av_xy, in1=a2_xy,
                                        op=OP.max)
                nc.vector.tensor_tensor(out=g_z, in0=av_z, in1=a2_z,
                                        op=OP.max)

                # --- squared center distance (ACT) ---
                # d2/o2 share one [P,4,T] tile (d2x,d2y,o2x,o2y) so the diag
                # sums below pair into two [2T]-mode adds
                Q = tmp.tile([P, 4, T], BF16, tag="Q4", name="Q")
                Z = tmp.tile([P, 2, T], BF16, tag="Z2", name="Z")
                nc.scalar.activation(out=Q[:, 0:2], in_=dc_xy, func=AF.Square)
                nc.scalar.activation(out=Z[:, 0:1], in_=dc_z, func=AF.Square)

                # --- 2*inter = relu(u-g); 2*outer = u+g ---
                ti_xy = t2("ti_xy")
                to_xy = t2("to_xy")
                ti_z = t1("ti_z")
                to_z = t1("to_z")
                nc.vector.tensor_sub(ti_xy, u_xy, g_xy)
                nc.vector.tensor_add(to_xy, u_xy, g_xy)
                nc.vector.tensor_sub(ti_z, u_z, g_z)
                nc.vector.tensor_add(to_z, u_z, g_z)
                if k == K - 1:
                    # final tile: relus on VEC so ACT reaches the o2 squares
                    # sooner (no next tile hides that latency)
                    nc.vector.tensor_scalar_max(ti_xy, ti_xy, 0.0)
                    nc.vector.tensor_scalar_max(ti_z, ti_z, 0.0)
                    nc.vector.tensor_scalar_max(to_xy[:, 1:2],
                                                to_xy[:, 1:2], 0.0)
                else:
                    nc.scalar.activation(out=ti_xy, in_=ti_xy, func=AF.Relu)
                    nc.scalar.activation(out=ti_z, in_=ti_z, func=AF.Relu)
                    # only outer_y can be negative
                    nc.scalar.activation(out=to_xy[:, 1:2], in_=to_xy[:, 1:2],
                                         func=AF.Relu)

                # deferred prefetch: issue the next tile's DMA only now so
                # it never steals HBM bandwidth from the current tile's load
                if k + 1 < K:
                    raw_nx = rawp.tile([P, C, T], BF16, tag="raw", name="raw")
                    nc.sync.dma_start(out=raw_nx, in_=abv[k + 1])

                # --- outer squares (ACT) ---
                nc.scalar.activation(out=Q[:, 2:4], in_=to_xy, func=AF.Square)
                nc.scalar.activation(out=Z[:, 1:2], in_=to_z, func=AF.Square)

                # next tile's trig: queued last on ACT so it never delays
                # this tile's a2/d2/o2, but still lands before the next
                # tile's products need it
                if raw_nx is not None:
                    sn_nx, cs_nx = trig(raw_nx)

                # --- reduce over dims ---
                I = t1("I")
                nc.vector.tensor_mul(I, ti_xy[:, 0:1], ti_xy[:, 1:2])
                nc.vector.tensor_mul(I, I, ti_z)
                # (idiag, O4) = (d2x+d2y, o2x+o2y) + (d2z, o2z): two
                # paired [2T] adds instead of four [T] adds
                dO = tmp.tile([P, 2, T], BF16, tag="dO2", name="dO")
                Qv = Q.rearrange("p (a b) t -> p b a t", a=2, b=2)
                nc.vector.tensor_add(dO, Qv[:, 0], Qv[:, 1])
                nc.vector.tensor_add(dO, dO, Z)
                idiag = dO[:, 0:1]
                O4 = dO[:, 1:2]

                # --- -diou = (4*idiag*U8 - I*O4) / (U8*O4) ---
                U8 = t1("U8")
                nc.vector.tensor_scalar_mul(U8, S, 8.0)
                nc.vector.tensor_sub(U8, U8, I)
                last = k == K - 1
                if last:
                    # final tile: keep the division chain off ACT so the
                    # closing serial tail never waits on the ACT queue
                    UO = f1("UO")
                    nc.vector.tensor_mul(UO, U8, O4)
                else:
                    UOb = tmp.tile([P, 1, T], BF16, tag="B19", name="UOb")
                    nc.vector.tensor_mul(UOb, U8, O4)
                    UO = f1("UO")
                    nc.scalar.copy(UO, UOb)
                rcp = f1("rcp")
                nc.vector.reciprocal_approx_fast(out=rcp, in_=UO)
                # ACT downcast so negd is a 2x bf16 multiply; the IO/DU/negnum
                # ops cover the cast latency (the last tile's ACT queue is
                # idle by tail-time, so this is safe there too)
                rcpb = tmp.tile([P, 1, T], BF16, tag="B17", name="rcpb")
                nc.scalar.copy(rcpb, rcp)
                IO = t1("IO")
                DU = t1("DU")
                nc.vector.tensor_mul(IO, I, O4)
                nc.vector.tensor_mul(DU, idiag, U8)
                negnum = t1("negnum")
                nc.vector.tensor_scalar_mul(negnum, DU, 4.0)
                nc.vector.tensor_sub(negnum, negnum, IO)
                negd = t1("negd")
                nc.vector.tensor_mul(negd, negnum, rcpb)
                # clip to [-1, 1] then row-sum into acc[:, k]; bf16 keeps the
                # clip at 4x and the reduce at 2x, accumulation is fp32
                dump = t1("dump")
                if last:
                    # split the closing clip so the first half's output DMA
                    # overlaps the second half's clip
                    H = T // 2
                    nc.vector.tensor_scalar(out=dump[:, :, 0:H],
                                            in0=negd[:, :, 0:H], scalar1=1.0,
                                            scalar2=-1.0, op0=OP.min,
                                            op1=OP.max)
                    nc.sync.dma_start(out=dumpout[:, :, 0:H],
                                      in_=dump[:, :, 0:H])
                    nc.vector.tensor_scalar(out=dump[:, :, H:T],
                                            in0=negd[:, :, H:T], scalar1=1.0,
                                            scalar2=-1.0, op0=OP.min,
                                            op1=OP.max)
                else:
                    nc.vector.tensor_scalar(out=dump, in0=negd, scalar1=1.0,
                                            scalar2=-1.0, op0=OP.min,
                                            op1=OP.max)
                if not last:
                    # row-sum on the otherwise idle PE: per-column partition
                    # sums of the clipped tile accumulate into one PSUM bank
                    for j in range((T + P - 1) // P):
                        lo, hi = j * P, min((j + 1) * P, T)
                        nc.tensor.matmul(psum[0:hi - lo], dump[:, 0, lo:hi],
                                         ones, start=(k == 0 and j == 0),
                                         stop=(k == K - 2 and hi == T))
                    if k == K - 2:
                        # drain PSUM during the final tile's compute and ship
                        # it right away so no output DMA remains after the
                        # last tile's clip
                        nc.scalar.copy(acc[:, 0:1], psum)
                        nc.sync.dma_start(out=out[:, 0:1], in_=acc[:, 0:1])
                else:
                    # final tile: ship the clipped values out and let the
                    # host add this partial (cheaper than a serial 1x-mode
                    # reduce at the very end of the VEC stream)
                    nc.sync.dma_start(out=dumpout[:, :, T // 2:T],
                                      in_=dump[:, :, T // 2:T])

                if _DEBUG and k == 0:
                    for nm, tl in (("w2", raw[:, 0:2]), ("sn2", sn2),
                                   ("Ep2", Ep2), ("g_xy", g_xy),
                                   ("ti_xy", ti_xy), ("to_xy", to_xy),
                                   ("I", I),
                                   ("U8", U8), ("negnum", negnum),
                                   ("rcp", rcp), ("negd", negd),
                                   ("dump", dump)):
                        nc.sync.dma_start(out=dbg[nm], in_=tl)

                if raw_nx is not None:
                    raw, sn2, cs2 = raw_nx, sn_nx, cs_nx


    nc.compile()
    return nc


def _pack_planes(box_pred, box_gt):
    """Full [C, NPAD] bf16 plane-major array."""
    planes = np.empty((C, NPAD), dtype=ml_dtypes.bfloat16)
    src = (box_pred, box_gt)
    for i, (col, which) in enumerate(_PLANE_SRC):
        planes[i, :NREAL] = src[which][:, col].astype(ml_dtypes.bfloat16)
        planes[i, NREAL:] = _PLANE_PAD[i]
    return planes


def kernel(box_pred, box_gt):
    global _LAST
    box_pred = np.asarray(box_pred, dtype=np.float32)
    box_gt = np.asarray(box_gt, dtype=np.float32)
    n = box_pred.shape[0]
    assert n == NREAL, f"kernel hardcoded for N={NREAL}, got {n}"

    if "nc" not in _CACHE:
        _CACHE["nc"] = _build()
    nc = _CACHE["nc"]

    planes = _pack_planes(box_pred, box_gt)
    in_maps = []
    for c in range(NCORES):
        lo, hi = c * RCORE, (c + 1) * RCORE
        in_maps.append({"ab": np.ascontiguousarray(planes[:, lo:hi])})

    kw = dict(trace=True, trace_cores=[0]) if _TRACE else {}
    res = bass_utils.run_bass_kernel_spmd(nc, in_maps,
                                          core_ids=list(range(NCORES)), **kw)
    _LAST = res
    total_neg = sum(
        float(res.results[c]["out"].astype(np.float64).sum())
        + float(res.results[c]["dumpout"].astype(np.float64).sum())
        for c in range(NCORES)
    )
    loss = (NPAD + total_neg) / (NREAL + 1e-4)
    return np.float32(loss)


# revision 52
# speedup vs baseline: 1.0048x; 1.0048x over previous
"""DIoU regression loss on 8 Trainium2 NeuronCores (data-parallel).

loss = sum(1 - clip(diou(pred_i, gt_i), -1, 1)) / (N + 1e-4) over N=4M boxes.

Sharding: each core gets a contiguous slab of R = 128*T*K rows; the last
core's slab is padded with identical unit boxes whose diou == 1, so padded
rows contribute 0 to sum(1 - diou).

Layout: the host packs the 7 used columns of both boxes into a plane-major
bf16 array [14, RCORE] per core, so every on-chip operand is a unit-stride
[P, T] (or [P, 2, T]) bf16 slice -> DVE runs in its 2x packed mode and DMA
traffic is 14 MB/core instead of 36 MB.

Math (equivalent to the det3d corner-based reference): with full extents
  Ex = w*cos(r) + l*sin(r), Ey = l*cos(r) - w*sin(r), Ez = h
and per dim u = Ep+Eg, v = Ep-Eg, dc = cg-cp, g = max(|v|, |2dc|):
  2*inter_d = relu(u - g),   2*outer_d = u + g  (relu'd for y)
so with I = prod(relu(u-g)) = 8*inter_vol, U8 = 8*(volp+volg) - I = 8*union,
O4 = sum((u+g)^2) = 4*outer_diag, D = sum(dc^2):
  -diou = (4*D*U8 - I*O4) / (U8*O4)
Each core emits per-partition per-tile sums of clip(-diou, -1, 1); the host
combines: loss = (NPAD + total_negdiou) / (N + 1e-4).

Engine split: DVE does the binary chain; ACT (scalar) does sin/cos, the
abs/squares/relus and the f32<->bf16 casts around the reciprocal; the PE
row-sums the clipped tiles 0..K-2 into PSUM via ones-matmuls (drained to
SBUF during the last tile); the last tile's clipped values are DMA'd out
raw and summed by the host together with the 128 PSUM partials. GPSIMD is
deliberately unused: it shares an SBUF read port with the DVE's rd1, so any
GPSIMD streaming blocks every 2-input DVE op. sin/cos of tile k+1 are
computed during tile k's body (sn2/cs2 double-buffered) and the prefetch
DMA is deferred mid-body so it never competes with the current tile's load.
The last tile keeps its relus and the UO product on VEC so the closing
serial tail never head-of-line blocks on the ACT queue.
"""

import numpy as np
import ml_dtypes

import concourse.bacc as bacc
import concourse.mybir as mybir
import concourse.tile as tile
from concourse import bass_utils

P = 128          # SBUF partitions
T = 1304         # rows per partition per tile
K = 3            # tiles per core
NCORES = 8
RCORE = P * T * K            # 500,736 rows per core
NPAD = RCORE * NCORES        # 4,005,888
NREAL = 4_000_000
C = 14                       # planes
BF16 = mybir.dt.bfloat16
F32 = mybir.dt.float32
HALF_PI = float(np.pi / 2)

AF = mybir.ActivationFunctionType
OP = mybir.AluOpType

# plane order: w_p w_g l_p l_g h_p h_g r_p r_g x_p y_p x_g y_g z_p z_g
_PLANE_SRC = [(3, 0), (3, 1), (4, 0), (4, 1), (5, 0), (5, 1), (6, 0), (6, 1),
              (0, 0), (1, 0), (0, 1), (1, 1), (2, 0), (2, 1)]
_PLANE_PAD = np.array([1, 1, 1, 1, 1, 1, 0, 0, 0, 0, 0, 0, 0, 0],
                      dtype=np.float32)

_CACHE = {}
_TRACE = False
_LAST = None
_DEBUG = False


def _build():
    nc = bacc.Bacc("TRN2", target_bir_lowering=False, debug=False,
                   num_devices=NCORES)
    ab = nc.dram_tensor("ab", [C, RCORE], BF16, kind="ExternalInput").ap()
    out = nc.dram_tensor("out", [P, 2], F32, kind="ExternalOutput").ap()
    dumpout = nc.dram_tensor("dumpout", [P, 1, T], BF16,
                             kind="ExternalOutput").ap()
    dbg = {}
    if _DEBUG:
        for nm in ("w2", "sn2", "Ep2", "g_xy", "ti_xy", "to_xy"):
            dbg[nm] = nc.dram_tensor(f"dbg_{nm}", [P, 2, T], BF16,
                                     kind="ExternalOutput").ap()
        for nm in ("I", "idiag", "O4", "U8", "negnum", "negd", "dump"):
            dbg[nm] = nc.dram_tensor(f"dbg_{nm}", [P, 1, T], BF16,
                                     kind="ExternalOutput").ap()
        for nm in ("rcp",):
            dbg[nm] = nc.dram_tensor(f"dbg_{nm}", [P, 1, T], F32,
                                     kind="ExternalOutput").ap()

    # [k][P, C, T]: plane c of tile k, partition p starts at
    # c*RCORE + k*P*T + p*T
    abv = ab.rearrange("c (k p t) -> k p c t", p=P, t=T)

    with tile.TileContext(nc) as tc:
        with (
            tc.tile_pool(name="raw", bufs=2) as rawp,
            tc.tile_pool(name="tmp", bufs=1) as tmp,
            tc.tile_pool(name="one", bufs=1) as one,
            tc.psum_pool(name="ps", bufs=1) as psp,
        ):
            psum = psp.tile([P, 1], F32, tag="psum", name="psum")
            ones = one.tile([P, 1], BF16, tag="ones", name="ones")
            nc.vector.memset(ones, 1.0)
            halfpi = one.tile([P, 1], F32, tag="halfpi", name="halfpi")
            nc.vector.memset(halfpi, HALF_PI)
            acc = one.tile([P, 2], F32, tag="acc", name="acc")

            # physical buffer tags; a tag may host several short-lived
            # logical tiles per iteration (WAR handled by the dep tracker).
            # sn2/cs2 are double-buffered for the trig software pipeline.
            _ALIAS = {
                "sn2": "A0", "cs2": "A1", "wl2": "A2", "vol2": "A3",
                "wc2": "A4", "ls2": "A5", "ws2": "A12", "lc2": "A7",
                "Ep2": "A8", "Eg2": "A9",
                "u_xy": "A4", "v_xy": "A5", "dc_xy": "A6", "g_xy": "A7",
                "a2_xy": "A13", "nv_xy": "A12", "ti_xy": "A8", "to_xy": "A9",
                "d2_xy": "A10", "o2_xy": "A11",
                "wl2": "A2", "vol2": "A3",
                "S": "B0", "u_z": "B1", "v_z": "B2", "dc_z": "B3",
                "g_z": "B4", "a2_z": "B5", "nv_z": "B5", "ti_z": "B5",
                "to_z": "B6",
                "d2_z": "B16", "o2_z": "B4", "I": "B1", "idiag": "B3",
                "O4": "B6", "U8": "B5", "IO": "B2", "DU": "B4",
                "negnum": "B7",
                "UO": "X0", "rcp": "X1", "negd": "X2", "dump": "X0",
            }
            _BUFS = {"A0": 2, "A1": 2}

            def t2(tag):
                t = _ALIAS[tag]
                return tmp.tile([P, 2, T], BF16, tag=t, name=tag,
                                bufs=_BUFS.get(t, 1))

            def t1(tag):
                t = _ALIAS[tag]
                return tmp.tile([P, 1, T], BF16, tag=t, name=tag,
                                bufs=_BUFS.get(t, 1))

            def f1(tag):
                t = _ALIAS[tag]
                return tmp.tile([P, 1, T], F32, tag=t, name=tag,
                                bufs=_BUFS.get(t, 1))

            def trig(raw_tile):
                sn2 = t2("sn2")
                cs2 = t2("cs2")
                r2 = raw_tile[:, 6:8]
                nc.scalar.activation(out=sn2, in_=r2, func=AF.Sin)
                # cos(r) = sin(pi/2 - r); keeps the arg inside the Sin
                # spline's domain.
                nc.scalar.activation(out=cs2, in_=r2, func=AF.Sin,
                                     bias=halfpi, scale=-1.0)
                return sn2, cs2

            # prologue: tile 0 DMA (sizes+trig planes first, centers second)
            raw = rawp.tile([P, C, T], BF16, tag="raw", name="raw")
            nc.sync.dma_start(out=raw[:, 0:4], in_=abv[0][:, 0:4])
            nc.sync.dma_start(out=raw[:, 4:6], in_=abv[0][:, 4:6])
            nc.sync.dma_start(out=raw[:, 6:8], in_=abv[0][:, 6:8])
            nc.sync.dma_start(out=raw[:, 8:14], in_=abv[0][:, 8:14])
            sn2, cs2 = trig(raw)

            for k in range(K):
                raw_nx = None

                w2 = raw[:, 0:2]
                l2 = raw[:, 2:4]
                h2 = raw[:, 4:6]
                cp_xy = raw[:, 8:10]
                cg_xy = raw[:, 10:12]
                zp = raw[:, 12:13]
                zg = raw[:, 13:14]

                if k == 0:
                    # tile 0: volumes first — they need no trig, so VEC works
                    # through them while the r-DMA/sin chain completes
                    wl2 = t2("wl2")
                    vol2 = t2("vol2")
                    S = t1("S")
                    nc.vector.tensor_mul(wl2, w2, l2)
                    nc.vector.tensor_mul(vol2, wl2, h2)
                    nc.vector.tensor_add(S, vol2[:, 0:1], vol2[:, 1:2])

                # --- rotated extents (VEC); sin-consumers first so the
                # products can start as soon as sin lands ---
                wc2 = t2("wc2")
                ls2 = t2("ls2")
                ws2 = t2("ws2")
                lc2 = t2("lc2")
                nc.vector.tensor_mul(ls2, l2, sn2)
                nc.vector.tensor_mul(ws2, w2, sn2)
                nc.vector.tensor_mul(wc2, w2, cs2)
                nc.vector.tensor_mul(lc2, l2, cs2)

                # --- center deltas (ACT computes |2dc| and dc^2 while VEC
                # chews on extents) ---
                dc_xy = t2("dc_xy")
                dc_z = t1("dc_z")
                nc.vector.tensor_sub(dc_xy, cg_xy, cp_xy)
                nc.vector.tensor_sub(dc_z, zg, zp)
                a2_xy = t2("a2_xy")
                a2_z = t1("a2_z")
                nc.scalar.activation(out=a2_xy, in_=dc_xy, func=AF.Abs,
                                     scale=2.0)
                nc.scalar.activation(out=a2_z, in_=dc_z, func=AF.Abs,
                                     scale=2.0)

                Ep2 = t2("Ep2")   # (Ex_p, Ey_p)
                Eg2 = t2("Eg2")
                nc.vector.tensor_add(Ep2[:, 0:1], wc2[:, 0:1], ls2[:, 0:1])
                nc.vector.tensor_sub(Ep2[:, 1:2], lc2[:, 0:1], ws2[:, 0:1])
                nc.vector.tensor_add(Eg2[:, 0:1], wc2[:, 1:2], ls2[:, 1:2])
                nc.vector.tensor_sub(Eg2[:, 1:2], lc2[:, 1:2], ws2[:, 1:2])

                # --- per-dim v first so ACT's |v| overlaps the u adds ---
                v_xy = t2("v_xy")
                v_z = t1("v_z")
                nc.vector.tensor_sub(v_xy, Ep2, Eg2)
                nc.vector.tensor_sub(v_z, h2[:, 0:1], h2[:, 1:2])
                av_xy = t2("nv_xy")
                av_z = tmp.tile([P, 1, T], BF16, tag="B18", name="av_z")
                nc.scalar.activation(out=av_xy, in_=v_xy, func=AF.Abs)
                nc.scalar.activation(out=av_z, in_=v_z, func=AF.Abs)
                u_xy = t2("u_xy")
                u_z = t1("u_z")
                nc.vector.tensor_add(u_xy, Ep2, Eg2)
                nc.vector.tensor_add(u_z, h2[:, 0:1], h2[:, 1:2])

                if k > 0:
                    # volumes as additional cover for the |v| latency
                    # (VEC-resident: GPSIMD would contend for the DVE's
                    # second SBUF read port and block every 2-input op)
                    wl2 = t2("wl2")
                    vol2 = t2("vol2")
                    S = t1("S")
                    nc.vector.tensor_mul(wl2, w2, l2)
                    nc.vector.tensor_mul(vol2, wl2, h2)
                    nc.vector.tensor_add(S, vol2[:, 0:1], vol2[:, 1:2])

                # --- g = max(|v|, |2dc|) ---
                g_xy = t2("g_xy")
                g_z = t1("g_z")
                nc.vector.tensor_tensor(out=g_xy, in0=av_xy, in1=a2_xy,
                                        op=OP.max)
                nc.vector.tensor_tensor(out=g_z, in0=av_z, in1=a2_z,
                                        op=OP.max)

                # --- squared center distance (ACT) ---
                d2_xy = t2("d2_xy")
                d2_z = t1("d2_z")
                nc.scalar.activation(out=d2_xy, in_=dc_xy, func=AF.Square)
                nc.scalar.activation(out=d2_z, in_=dc_z, func=AF.Square)

                # --- 2*inter = relu(u-g); 2*outer = u+g ---
                ti_xy = t2("ti_xy")
                to_xy = t2("to_xy")
                ti_z = t1("ti_z")
                to_z = t1("to_z")
                nc.vector.tensor_sub(ti_xy, u_xy, g_xy)
                nc.vector.tensor_add(to_xy, u_xy, g_xy)
                nc.vector.tensor_sub(ti_z, u_z, g_z)
                nc.vector.tensor_add(to_z, u_z, g_z)
                if k == K - 1:
                    # final tile: relus on VEC so ACT reaches the o2 squares
                    # sooner (no next tile hides that latency)
                    nc.vector.tensor_scalar_max(ti_xy, ti_xy, 0.0)
                    nc.vector.tensor_scalar_max(ti_z, ti_z, 0.0)
                    nc.vector.tensor_scalar_max(to_xy[:, 1:2],
                                                to_xy[:, 1:2], 0.0)
                else:
                    nc.scalar.activation(out=ti_xy, in_=ti_xy, func=AF.Relu)
                    nc.scalar.activation(out=ti_z, in_=ti_z, func=AF.Relu)
                    # only outer_y can be negative
                    nc.scalar.activation(out=to_xy[:, 1:2], in_=to_xy[:, 1:2],
                                         func=AF.Relu)

                # deferred prefetch: issue the next tile's DMA only now so
                # it never steals HBM bandwidth from the current tile's load
                if k + 1 < K:
                    raw_nx = rawp.tile([P, C, T], BF16, tag="raw", name="raw")
                    nc.sync.dma_start(out=raw_nx, in_=abv[k + 1])

                # --- outer squares (ACT) ---
                o2_xy = t2("o2_xy")
                o2_z = t1("o2_z")
                nc.scalar.activation(out=o2_xy, in_=to_xy, func=AF.Square)
                nc.scalar.activation(out=o2_z, in_=to_z, func=AF.Square)

                # next tile's trig: queued last on ACT so it never delays
                # this tile's a2/d2/o2, but still lands before the next
                # tile's products need it
                if raw_nx is not None:
                    sn_nx, cs_nx = trig(raw_nx)

                # --- reduce over dims ---
                I = t1("I")
                nc.vector.tensor_mul(I, ti_xy[:, 0:1], ti_xy[:, 1:2])
                nc.vector.tensor_mul(I, I, ti_z)
                idiag = t1("idiag")
                nc.vector.tensor_add(idiag, d2_xy[:, 0:1], d2_xy[:, 1:2])
                nc.vector.tensor_add(idiag, idiag, d2_z)
                O4 = t1("O4")
                nc.vector.tensor_add(O4, o2_xy[:, 0:1], o2_xy[:, 1:2])
                nc.vector.tensor_add(O4, O4, o2_z)

                # --- -diou = (4*idiag*U8 - I*O4) / (U8*O4) ---
                U8 = t1("U8")
                nc.vector.tensor_scalar_mul(U8, S, 8.0)
                nc.vector.tensor_sub(U8, U8, I)
                last = k == K - 1
                if last:
                    # final tile: keep the division chain off ACT so the
                    # closing serial tail never waits on the ACT queue
                    UO = f1("UO")
                    nc.vector.tensor_mul(UO, U8, O4)
                else:
                    UOb = tmp.tile([P, 1, T], BF16, tag="B19", name="UOb")
                    nc.vector.tensor_mul(UOb, U8, O4)
                    UO = f1("UO")
                    nc.scalar.copy(UO, UOb)
                rcp = f1("rcp")
                nc.vector.reciprocal_approx_fast(out=rcp, in_=UO)
                # ACT downcast so negd is a 2x bf16 multiply; the IO/DU/negnum
                # ops cover the cast latency (the last tile's ACT queue is
                # idle by tail-time, so this is safe there too)
                rcpb = tmp.tile([P, 1, T], BF16, tag="B17", name="rcpb")
                nc.scalar.copy(rcpb, rcp)
                IO = t1("IO")
                DU = t1("DU")
                nc.vector.tensor_mul(IO, I, O4)
                nc.vector.tensor_mul(DU, idiag, U8)
                negnum = t1("negnum")
                nc.vector.tensor_scalar_mul(negnum, DU, 4.0)
                nc.vector.tensor_sub(negnum, negnum, IO)
                negd = t1("negd")
                nc.vector.tensor_mul(negd, negnum, rcpb)
                # clip to [-1, 1] then row-sum into acc[:, k]; bf16 keeps the
                # clip at 4x and the reduce at 2x, accumulation is fp32
                dump = t1("dump")
                if last:
                    # split the closing clip so the first half's output DMA
                    # overlaps the second half's clip
                    H = T // 2
                    nc.vector.tensor_scalar(out=dump[:, :, 0:H],
                                            in0=negd[:, :, 0:H], scalar1=1.0,
                                            scalar2=-1.0, op0=OP.min,
                                            op1=OP.max)
                    nc.sync.dma_start(out=dumpout[:, :, 0:H],
                                      in_=dump[:, :, 0:H])
                    nc.vector.tensor_scalar(out=dump[:, :, H:T],
                                            in0=negd[:, :, H:T], scalar1=1.0,
                                            scalar2=-1.0, op0=OP.min,
                                            op1=OP.max)
                else:
                    nc.vector.tensor_scalar(out=dump, in0=negd, scalar1=1.0,
                                            scalar2=-1.0, op0=OP.min,
                                            op1=OP.max)
                if not last:
                    # row-sum on the otherwise idle PE: per-column partition
                    # sums of the clipped tile accumulate into one PSUM bank
                    for j in range((T + P - 1) // P):
                        lo, hi = j * P, min((j + 1) * P, T)
                        nc.tensor.matmul(psum[0:hi - lo], dump[:, 0, lo:hi],
                                         ones, start=(k == 0 and j == 0),
                                         stop=(k == K - 2 and hi == T))
                    if k == K - 2:
                        # drain PSUM during the final tile's compute and ship
                        # it right away so no output DMA remains after the
                        # last tile's clip
                        nc.scalar.copy(acc[:, 0:1], psum)
                        nc.sync.dma_start(out=out[:, 0:1], in_=acc[:, 0:1])
                else:
                    # final tile: ship the clipped values out and let the
                    # host add this partial (cheaper than a serial 1x-mode
                    # reduce at the very end of the VEC stream)
                    nc.sync.dma_start(out=dumpout[:, :, T // 2:T],
                                      in_=dump[:, :, T // 2:T])

                if _DEBUG and k == 0:
                    for nm, tl in (("w2", raw[:, 0:2]), ("sn2", sn2),
                                   ("Ep2", Ep2), ("g_xy", g_xy),
                                   ("ti_xy", ti_xy), ("to_xy", to_xy),
                                   ("I", I), ("idiag", idiag), ("O4", O4),
                                   ("U8", U8), ("negnum", negnum),
                                   ("rcp", rcp), ("negd", negd),
                                   ("dump", dump)):
                        nc.sync.dma_start(out=dbg[nm], in_=tl)

                if raw_nx is not None:
                    raw, sn2, cs2 = raw_nx, sn_nx, cs_nx


    nc.compile()
    return nc


def _pack_planes(box_pred, box_gt):
    """Full [C, NPAD] bf16 plane-major array."""
    planes = np.empty((C, NPAD), dtype=ml_dtypes.bfloat16)
    src = (box_pred, box_gt)
    for i, (col, which) in enumerate(_PLANE_SRC):
        planes[i, :NREAL] = src[which][:, col].astype(ml_dtypes.bfloat16)
        planes[i, NREAL:] = _PLANE_PAD[i]
    return planes


def kernel(box_pred, box_gt):
    global _LAST
    box_pred = np.asarray(box_pred, dtype=np.float32)
    box_gt = np.asarray(box_gt, dtype=np.float32)
    n = box_pred.shape[0]
    assert n == NREAL, f"kernel hardcoded for N={NREAL}, got {n}"

    if "nc" not in _CACHE:
        _CACHE["nc"] = _build()
    nc = _CACHE["nc"]

    planes = _pack_planes(box_pred, box_gt)
    in_maps = []
    for c in range(NCORES):
        lo, hi = c * RCORE, (c + 1) * RCORE
        in_maps.append({"ab": np.ascontiguousarray(planes[:, lo:hi])})

    kw = dict(trace=True, trace_cores=[0]) if _TRACE else {}
    res = bass_utils.run_bass_kernel_spmd(nc, in_maps,
                                          core_ids=list(range(NCORES)), **kw)
    _LAST = res
    total_neg = sum(
        float(res.results[c]["out"].astype(np.float64).sum())
        + float(res.results[c]["dumpout"].astype(np.float64).sum())
        for c in range(NCORES)
    )
    loss = (NPAD + total_neg) / (NREAL + 1e-4)
    return np.float32(loss)
